# revision 9
# baseline (speedup 1.0000x reference)
"""GraphSAGE 2-layer GNN, fully on-device on 8 Trainium2 NeuronCores.

Node-parallel sharding per the hint: each core owns 12500 dst nodes
(padded to 12800).  The full (padded) feature table X_pad [102400, 128]
bf16 is replicated into every core's HBM; each core dma_gathers the
messages for its own edges (int16 gather indices => the table is split
into 4 chunks of 25600 rows, edges bucketed per (dst-block, chunk)).

Segment-mean aggregation is done on the tensor engine: for each group
of 128 dst-sorted edges an indicator matrix ind[e, n] =
(dstl[e] == n) * invc[e] is built on DVE (iota-const compared against a
stride-0-broadcast dstl column, scaled by 1/deg), and
psum_agg[feat, node] accumulates matmul(lhsT=msgs[e,feat],
rhs=ind[e,node]) over the block's groups.  The layer output
h = relu(agg@Wl + x@Wr + b) is computed per 128-node block with three
more matmuls (bias via a K=1 ones x brow matmul) in BOTH orientations
during layer 1: [node, fo] (written to HBM for the inter-layer
AllGather) and [fo, node] (kept resident in SBUF as the layer-2 self
term).  h1 slabs are AllGather'd across the 8 cores in 5 pipelined
chunks; layer 2 gathers its messages from the gathered H1_pad.  The two
heads run on DVE (tensor_tensor_reduce) + ACT (sigmoid) per block.

One Bass program, compiled once, SPMD on cores 0-7.  All cores run the
IDENTICAL program; every data-dependent quantity (gather indices, local
dst ids, 1/deg) is input data.  Group counts are padded to a uniform
G_BC = max over (core, block, chunk) so the instruction stream is
core-independent.

Host work is limited to one-time edge bucketing (argsort + scatters)
and dtype conversion.  A pure-numpy fallback path is kept in case the
device path raises.
"""

import os
import numpy as np
import ml_dtypes

BF16 = ml_dtypes.bfloat16

# ---------------- configuration ----------------


class Cfg:
    def __init__(self, n_nodes, n_edges, n_cores, own, ownp, nch, super_,
                 cch):
        self.N = n_nodes
        self.E = n_edges
        self.NC = n_cores
        self.OWN = own                 # real nodes per core
        self.OWNP = ownp               # padded nodes per core (mult of 128)
        self.D = 128
        self.PADN = ownp * n_cores     # padded feature-table rows
        self.NCH = nch                 # gather source chunks
        self.CHROWS = self.PADN // nch
        assert self.CHROWS <= 32767 and self.PADN % nch == 0
        assert ownp % self.CHROWS == 0 or self.CHROWS % ownp == 0
        self.BLKS = ownp // 128        # dst blocks per core
        self.SUPER = super_            # blocks per super (psum-resident)
        assert self.BLKS % super_ == 0
        self.NSUP = self.BLKS // super_
        self.CCH = cch                 # supers per collective chunk
        assert self.NSUP % cch == 0
        self.NCOLL = self.NSUP // cch


CFG = Cfg(n_nodes=100000, n_edges=1600000, n_cores=8, own=12500,
          ownp=12800, nch=4, super_=5, cch=4)

LAST_TRACE = None     # BassKernelResults of the traced launch (test use)
LAST_LAUNCH_S = None  # wall seconds of the device launch (test use)


def _trace_available():
    try:
        from antenv.axon_hooks import get_axon_ntff_profile_hook  # noqa
        return True
    except Exception:
        return False


# ---------------- host-side edge prep ----------------


def _prep(cfg, edge_index, x):
    """Bucket edges, build all per-core device inputs. Returns dict."""
    src = np.asarray(edge_index[0], np.int64)
    dst = np.asarray(edge_index[1], np.int64)
    OWN, OWNP, NCH, D = cfg.OWN, cfg.OWNP, cfg.NCH, cfg.D

    core = dst // OWN
    iblk = (dst % OWN) // 128                   # block within core
    dstl = (dst % OWN) % 128                    # local id within block
    # chunk-major padded table row (collective outputs must be contiguous):
    # row(k, i) = (i // CR) * NC * CR + k * CR + (i % CR)
    CR = cfg.CCH * cfg.SUPER * 128
    sk = src // OWN
    si = src % OWN
    r = (si // CR) * (cfg.NC * CR) + sk * CR + (si % CR)
    ch = r // cfg.CHROWS
    lidx = r % cfg.CHROWS

    cnt = np.bincount(dst, minlength=cfg.N)
    invc_node = (1.0 / np.maximum(cnt, 1)).astype(np.float32)
    invc_e = invc_node[dst]

    nkey = (core * cfg.BLKS + iblk) * NCH + ch  # (core, block, chunk)
    nkeys = cfg.NC * cfg.BLKS * NCH
    counts = np.bincount(nkey, minlength=nkeys)
    gbc = max(1, int(-(-counts.max() // 128)))  # ceil
    slot = gbc * 128

    order = np.argsort(nkey, kind="stable")
    starts = np.zeros(nkeys, np.int64)
    starts[1:] = np.cumsum(counts)[:-1]
    rank = np.arange(cfg.E, dtype=np.int64) - starts[nkey[order]]
    pos = nkey[order] * slot + rank

    tot = nkeys * slot
    idx_pad = np.zeros(tot, np.int16)
    dstl_pad = np.full(tot, -1, np.int16)
    invc_pad = np.zeros(tot, np.float32)
    idx_pad[pos] = lidx[order].astype(np.int16)
    dstl_pad[pos] = dstl[order].astype(np.int16)
    invc_pad[pos] = invc_e[order]

    # [NC, BLKS, NCH, slot] -> gather streams per (core, sup, chunk):
    # stream = blocks of the super concatenated: [SUPER * slot]
    def to_streams(a):
        a = a.reshape(cfg.NC, cfg.NSUP, cfg.SUPER, NCH, slot)
        return np.ascontiguousarray(a.transpose(0, 1, 3, 2, 4))
        # -> [NC, NSUP, NCH, SUPER, slot]

    idx_s = to_streams(idx_pad)
    ns = cfg.SUPER * slot                       # idxs per gather
    # pack16: idx i at [i%16, i//16]
    p16 = idx_s.reshape(cfg.NC, cfg.NSUP, NCH, ns // 16, 16)
    p16 = p16.transpose(0, 4, 1, 2, 3).reshape(cfg.NC, 16, -1)
    idx_all = np.zeros((cfg.NC, 128, p16.shape[2]), np.int16)
    idx_all[:, :16, :] = p16

    # dstl/invc: [NC, NSUP, NCH, SUPER, gbc, 128] -> [NC, 128, NGTOT]
    def to_cols(a, dt):
        a = a.reshape(cfg.NC, cfg.NSUP, cfg.SUPER, NCH, gbc, 128)
        a = a.transpose(0, 5, 1, 3, 2, 4)       # [NC,128,NSUP,NCH,SUPER,gbc]
        return np.ascontiguousarray(a.reshape(cfg.NC, 128, -1)).astype(dt)

    dstl_all = to_cols(dstl_pad, np.int16)
    invc_all = to_cols(invc_pad, BF16)

    x_bf = np.asarray(x, np.float32).astype(BF16)
    xpad = np.zeros((cfg.PADN, D), BF16)
    xT1 = np.zeros((cfg.NC, D, OWNP), BF16)
    iv = np.arange(OWN, dtype=np.int64)
    rows_own = (iv // CR) * (cfg.NC * CR) + iv % CR
    for k in range(cfg.NC):
        xe = x_bf[k * OWN:(k + 1) * OWN]
        xpad[rows_own + k * CR] = xe
        xT1[k, :, :OWN] = xe.T

    iota = np.broadcast_to(
        np.arange(128, dtype=np.int16)[None, None, :],
        (128, cfg.SUPER * gbc, 128)).reshape(128, -1)
    iota = np.ascontiguousarray(iota)

    return dict(gbc=gbc, idx_all=idx_all, dstl_all=dstl_all,
                invc_all=invc_all, xpad=xpad, xT1=xT1, iota=iota)


# ---------------- bass program ----------------


def _build(cfg, gbc, bp_val, bd_val):
    import concourse.bass as bass
    import concourse.tile as tile
    import concourse.mybir as mybir
    from concourse import bacc

    f32 = mybir.dt.float32
    bf16 = mybir.dt.bfloat16
    i16 = mybir.dt.int16
    AOT = mybir.AluOpType
    ACT_F = mybir.ActivationFunctionType

    D, NCH, SUPER, NSUP = cfg.D, cfg.NCH, cfg.SUPER, cfg.NSUP
    slot = gbc * 128
    ns = SUPER * slot                    # idxs per gather
    ngsc = SUPER * gbc                   # groups per (s, c)
    ngtot = NSUP * NCH * ngsc
    wcols = ns // 16                     # idx cols per gather
    crows_c = cfg.NCOLL * 0 + cfg.CCH * SUPER * 128   # nodes per collective

    nc = bacc.Bacc("TRN2", target_bir_lowering=False, debug=False)
    xp_d = nc.dram_tensor("xpad", [cfg.PADN, D], bf16, kind="ExternalInput")
    xt1_d = nc.dram_tensor("xT1", [D, cfg.OWNP], bf16, kind="ExternalInput")
    idx_d = nc.dram_tensor("idx_all", [128, NSUP * NCH * wcols], i16,
                           kind="ExternalInput")
    dstl_d = nc.dram_tensor("dstl_all", [128, ngtot], i16,
                            kind="ExternalInput")
    invc_d = nc.dram_tensor("invc_all", [128, ngtot], bf16,
                            kind="ExternalInput")
    iota_d = nc.dram_tensor("iota_c", [128, ngsc * 128], i16,
                            kind="ExternalInput")
    w_d = {}
    for w in ("wl1", "wr1", "wl2", "wr2"):
        w_d[w] = nc.dram_tensor(w, [D, D], bf16, kind="ExternalInput")
    brow1_d = nc.dram_tensor("brow1", [1, D], bf16, kind="ExternalInput")
    brow2_d = nc.dram_tensor("brow2", [1, D], bf16, kind="ExternalInput")
    ones_d = nc.dram_tensor("ones_r", [1, D], bf16, kind="ExternalInput")
    wpb_d = nc.dram_tensor("wp_b", [128, D], f32, kind="ExternalInput")
    wdb_d = nc.dram_tensor("wd_b", [128, D], f32, kind="ExternalInput")
    pd_d = nc.dram_tensor("pd", [cfg.OWNP, 2], f32, kind="ExternalOutput")

    h1own = [nc.dram_tensor(f"h1own{q}", [crows_c, D], bf16)
             for q in range(cfg.NCOLL)]
    h1g = nc.dram_tensor("h1gath", [cfg.PADN, D], bf16,
                         addr_space="Shared" if cfg.NC > 4 else "Local")

    with tile.TileContext(nc) as tc:
        with (
            tc.tile_pool(name="const", bufs=1) as cp,
            tc.tile_pool(name="idxp", bufs=4) as xp,
            tc.tile_pool(name="msgp", bufs=3) as mp,
            tc.tile_pool(name="indp", bufs=3) as ip,
            tc.tile_pool(name="sbp", bufs=4) as sp,
            tc.tile_pool(name="psa", bufs=1, space="PSUM") as pa_pool,
            tc.tile_pool(name="psh", bufs=1, space="PSUM") as ph_pool,
            tc.tile_pool(name="psh2", bufs=1, space="PSUM") as ph2_pool,
        ):
            ns_reg = nc.gpsimd.to_reg(ns)

            wt = {}
            for w in ("wl1", "wr1", "wl2", "wr2"):
                t = cp.tile([D, D], bf16, tag=w)
                nc.sync.dma_start(t[:], w_d[w][:])
                wt[w] = t
            brow1_t = cp.tile([1, D], bf16, tag="brow1")
            nc.sync.dma_start(brow1_t[:], brow1_d[:])
            brow2_t = cp.tile([1, D], bf16, tag="brow2")
            nc.sync.dma_start(brow2_t[:], brow2_d[:])
            ones_t = cp.tile([1, D], bf16, tag="ones")
            nc.sync.dma_start(ones_t[:], ones_d[:])
            wpb_t = cp.tile([128, D], f32, tag="wpb")
            nc.sync.dma_start(wpb_t[:], wpb_d[:])
            wdb_t = cp.tile([128, D], f32, tag="wdb")
            nc.sync.dma_start(wdb_t[:], wdb_d[:])
            iota_t = cp.tile([128, ngsc * 128], i16, tag="iota")
            nc.sync.dma_start(iota_t[:], iota_d[:])
            dstl_t = cp.tile([128, ngtot], i16, tag="dstl")
            nc.sync.dma_start(dstl_t[:], dstl_d[:])
            invc_t = cp.tile([128, ngtot], bf16, tag="invc")
            nc.sync.dma_start(invc_t[:], invc_d[:])
            xT1_t = cp.tile([D, cfg.OWNP], bf16, tag="xT1")
            nc.sync.dma_start(xT1_t[:], xt1_d[:])
            xT2_t = cp.tile([D, cfg.OWNP], bf16, tag="xT2")

            for layer in (1, 2):
                wl_t = wt["wl1"] if layer == 1 else wt["wl2"]
                wr_t = wt["wr1"] if layer == 1 else wt["wr2"]
                brow_t = brow1_t if layer == 1 else brow2_t
                xTs_t = xT1_t if layer == 1 else xT2_t
                src_d = xp_d if layer == 1 else h1g

                for s in range(NSUP):
                    msgt, indt = [], []
                    for c in range(NCH):
                        it = xp.tile([128, wcols], i16, tag="idx")
                        off = (s * NCH + c) * wcols
                        nc.sync.dma_start(it[:], idx_d[:, off:off + wcols])
                        m = mp.tile([128, SUPER * gbc, 128], bf16, tag="msg")
                        nc.gpsimd.dma_gather(
                            m[:],
                            src_d[c * cfg.CHROWS:(c + 1) * cfg.CHROWS, :],
                            it[:], num_idxs=ns, num_idxs_reg=ns_reg,
                            elem_size=D)
                        msgt.append(m)
                        gcol = (s * NCH + c) * ngsc
                        ind = ip.tile([128, ngsc * 128], bf16, tag="ind")
                        nc.vector.tensor_tensor(
                            ind[:], iota_t[:],
                            dstl_t[:, gcol:gcol + ngsc]
                            .to_broadcast((128, ngsc, 128)),
                            op=AOT.is_equal)
                        nc.vector.tensor_tensor(
                            ind[:], ind[:],
                            invc_t[:, gcol:gcol + ngsc]
                            .to_broadcast((128, ngsc, 128)),
                            op=AOT.mult)
                        indt.append(ind)

                    pa = [pa_pool.tile([128, 128], f32, tag=f"agg{b}",
                                       name=f"pa{b}")
                          for b in range(SUPER)]
                    for c in range(NCH):
                        for b in range(SUPER):
                            for g in range(gbc):
                                gg = b * gbc + g
                                nc.tensor.matmul(
                                    pa[b][:], msgt[c][:, gg, :],
                                    indt[c][:, gg * 128:gg * 128 + 128],
                                    start=(c == 0 and g == 0),
                                    stop=(c == NCH - 1 and g == gbc - 1))

                    for b in range(SUPER):
                        gblk = s * SUPER + b
                        aggsb = sp.tile([128, 128], bf16, tag="aggsb")
                        nc.vector.tensor_copy(aggsb[:], pa[b][:])
                        ph = ph_pool.tile([128, 128], f32, tag="ph")
                        nc.tensor.matmul(ph[:], aggsb[:], wl_t[:],
                                         start=True, stop=False)
                        nc.tensor.matmul(
                            ph[:], xTs_t[:, gblk * 128:gblk * 128 + 128],
                            wr_t[:], start=False, stop=False)
                        nc.tensor.matmul(ph[:], ones_t[:], brow_t[:],
                                         start=False, stop=True)
                        if layer == 1:
                            hsb = sp.tile([128, 128], bf16, tag="hsb")
                            nc.scalar.activation(hsb[:], ph[:], ACT_F.Relu,
                                                 bias=0.0, scale=1.0)
                            q = s // cfg.CCH
                            r0 = gblk * 128 - q * crows_c
                            nc.sync.dma_start(h1own[q][r0:r0 + 128, :],
                                              hsb[:])
                            ph2 = ph2_pool.tile([128, 128], f32, tag="ph2")
                            nc.tensor.matmul(ph2[:], wl_t[:], aggsb[:],
                                             start=True, stop=False)
                            nc.tensor.matmul(
                                ph2[:], wr_t[:],
                                xTs_t[:, gblk * 128:gblk * 128 + 128],
                                start=False, stop=False)
                            nc.tensor.matmul(ph2[:], brow_t[:], ones_t[:],
                                             start=False, stop=True)
                            nc.scalar.activation(
                                xT2_t[:, gblk * 128:gblk * 128 + 128],
                                ph2[:], ACT_F.Relu, bias=0.0, scale=1.0)
                        else:
                            hsb = sp.tile([128, 128], f32, tag="hsb2")
                            nc.scalar.activation(hsb[:], ph[:], ACT_F.Relu,
                                                 bias=0.0, scale=1.0)
                            junk = sp.tile([128, 128], f32, tag="junk")
                            pcol = sp.tile([128, 1], f32, tag="pcol")
                            zcol = sp.tile([128, 1], f32, tag="zcol")
                            nc.vector.tensor_tensor_reduce(
                                junk[:], hsb[:], wpb_t[:], 1.0, bp_val,
                                op0=AOT.mult, op1=AOT.add,
                                accum_out=pcol[:])
                            junk2 = sp.tile([128, 128], f32, tag="junk2")
                            nc.vector.tensor_tensor_reduce(
                                junk2[:], hsb[:], wdb_t[:], 1.0, bd_val,
                                op0=AOT.mult, op1=AOT.add,
                                accum_out=zcol[:])
                            dcol = sp.tile([128, 1], f32, tag="dcol")
                            nc.scalar.activation(dcol[:], zcol[:],
                                                 ACT_F.Sigmoid,
                                                 bias=0.0, scale=1.0)
                            pdsb = sp.tile([128, 2], f32, tag="pdsb")
                            nc.vector.tensor_sub(pdsb[:, 0:1], pcol[:],
                                                 dcol[:])
                            nc.vector.tensor_add(pdsb[:, 1:2], pcol[:],
                                                 dcol[:])
                            nc.sync.dma_start(
                                pd_d[gblk * 128:gblk * 128 + 128, :],
                                pdsb[:])

                    if layer == 1 and (s + 1) % cfg.CCH == 0:
                        q = s // cfg.CCH
                        out_ap = h1g[q * cfg.NC * crows_c:
                                     (q + 1) * cfg.NC * crows_c, :]
                        nc.gpsimd.collective_compute(
                            "AllGather",
                            mybir.AluOpType.bypass,
                            replica_groups=[list(range(cfg.NC))],
                            ins=[h1own[q][:, :].opt()],
                            outs=[out_ap.opt()],
                        )
    nc.compile()
    return nc


# ---------------- device path ----------------

_compiled = None       # (key, nc, prep)


def _device_kernel(cfg, x, edge_index, Wl1, Wr1, b1, Wl2, Wr2, b2,
                   Wp, bp, Wd, bd):
    global _compiled, LAST_TRACE, LAST_LAUNCH_S
    import time as _time
    from concourse.bass_utils import run_bass_kernel_spmd

    prep = _prep(cfg, edge_index, x)
    gbc = prep["gbc"]
    bp_val = float(np.asarray(bp).reshape(-1)[0])
    bd_val = float(np.asarray(bd).reshape(-1)[0])

    key = (gbc, bp_val, bd_val)
    if _compiled is None or _compiled[0] != key:
        nc = _build(cfg, gbc, bp_val, bd_val)
        _compiled = (key, nc)
    nc = _compiled[1]

    def bfw(a):
        return np.ascontiguousarray(np.asarray(a, np.float32).astype(BF16))

    wp_b = np.ascontiguousarray(np.broadcast_to(
        np.asarray(Wp, np.float32).reshape(1, cfg.D), (128, cfg.D)))
    wd_b = np.ascontiguousarray(np.broadcast_to(
        np.asarray(Wd, np.float32).reshape(1, cfg.D), (128, cfg.D)))
    ones_r = np.ones((1, cfg.D), BF16)

    in_maps = []
    for k in range(cfg.NC):
        in_maps.append({
            "xpad": prep["xpad"],
            "xT1": prep["xT1"][k],
            "idx_all": prep["idx_all"][k],
            "dstl_all": prep["dstl_all"][k],
            "invc_all": prep["invc_all"][k],
            "iota_c": prep["iota"],
            "wl1": bfw(Wl1), "wr1": bfw(Wr1),
            "wl2": bfw(Wl2), "wr2": bfw(Wr2),
            "brow1": bfw(np.asarray(b1).reshape(1, cfg.D)),
            "brow2": bfw(np.asarray(b2).reshape(1, cfg.D)),
            "ones_r": ones_r,
            "wp_b": wp_b, "wd_b": wd_b,
        })

    trace = bool(os.environ.get("KERNEL_TRACE")) and _trace_available()
    _t0 = _time.time()
    res = run_bass_kernel_spmd(nc, in_maps, core_ids=list(range(cfg.NC)),
                               trace=trace)
    LAST_LAUNCH_S = _time.time() - _t0
    LAST_TRACE = res
    outs = res.results if hasattr(res, "results") else res
    pd = np.stack([np.asarray(o["pd"], np.float32) for o in outs])
    # [NC, OWNP, 2] -> real rows
    pd = pd[:, :cfg.OWN, :].reshape(cfg.NC * cfg.OWN, 2)[:cfg.N]
    lo = np.ascontiguousarray(pd[:, 0:1])
    hi = np.ascontiguousarray(pd[:, 1:2])
    return lo, hi


# ---------------- host fallback ----------------


def _host_kernel(x, edge_index, Wl1, Wr1, b1, Wl2, Wr2, b2, Wp, bp, Wd, bd):
    N = CFG.N
    x = np.asarray(x, np.float32)
    src = np.asarray(edge_index[0], np.int64)
    dst = np.asarray(edge_index[1], np.int64)
    order = np.argsort(dst, kind="stable")
    src_s, dst_s = src[order], dst[order]
    counts = np.bincount(dst_s, minlength=N)
    starts = np.zeros(N, np.int64)
    starts[1:] = np.cumsum(counts)[:-1]
    nz = counts > 0
    inv = (1.0 / np.maximum(counts[nz], 1)).astype(np.float32)

    def mean_agg(f):
        sums = np.add.reduceat(f[src_s], starts[nz], axis=0)
        agg = np.zeros((N, f.shape[1]), np.float32)
        agg[nz] = sums * inv[:, None]
        return agg

    def layer(f, Wl, Wr, b):
        return np.maximum(mean_agg(f) @ Wl + f @ Wr + b, 0.0)

    h = layer(x, np.asarray(Wl1, np.float32), np.asarray(Wr1, np.float32),
              np.asarray(b1, np.float32))
    h = layer(h, np.asarray(Wl2, np.float32), np.asarray(Wr2, np.float32),
              np.asarray(b2, np.float32))
    preds = h @ np.asarray(Wp, np.float32) + np.asarray(bp, np.float32)
    z = h @ np.asarray(Wd, np.float32) + np.asarray(bd, np.float32)
    diffs = 1.0 / (1.0 + np.exp(-z))
    return ((preds - diffs).astype(np.float32),
            (preds + diffs).astype(np.float32))


# ---------------- entry ----------------


def kernel(x, edge_index, Wl1, Wr1, b1, Wl2, Wr2, b2, Wp, bp, Wd, bd):
    if not os.environ.get("KERNEL_HOST_ONLY"):
        try:
            return _device_kernel(CFG, x, edge_index, Wl1, Wr1, b1,
                                  Wl2, Wr2, b2, Wp, bp, Wd, bd)
        except Exception:
            import traceback
            traceback.print_exc()
    return _host_kernel(x, edge_index, Wl1, Wr1, b1, Wl2, Wr2, b2,
                        Wp, bp, Wd, bd)
